# revision 26
# baseline (speedup 1.0000x reference)
"""Multi-head causal attention (B=2, S=2048, D=2048, 16 heads) on 8 TRN2 cores.

Sharding: 2-way batch parallel x 4-way head tensor-parallel (4 heads/core).
Each core computes q/k/v projections for its 4 heads, causal softmax
attention, and a partial o-projection; the host sums the 4 partials per batch.

v4: float32r matmuls in phase 1 and the o-projection (f32r self-loads its
stationary; 2-byte dtypes emit a per-matmul Ldweights the PE pays serially
on HW), fp16 for the attention phase + all resident activations (q/k/v stay
fully resident - no DRAM spill - and the out DMA is halved).  Host
pre-swizzles every input into a partition-major layout so each DMA is one
fat (8-32KB) contiguous DRAM run per SBUF partition:
  xh  [p, sc, dt, s] = x[b].T[(dt p), (sc s)]
  wqh [p, dt, j]     = wq[j0:j0+512].T[(dt p), j]   (same wkh, wvh)
  woh [p, hh, m]     = wo[:, j0:j0+512].T[(hh p), m]

On-chip dataflow (per core):
  phase 1: kT_all[dk,h,s], qT_all[dk,h,s], v_all[s,st,dv] all resident fp16.
           wv streamed per chunk in 4-dt pieces (dt-outer loop, 4 parallel
           PSUM banks) so the PE starts after ~1.3MB of DMA; wq/wk resident,
           k/q projections s-outer, psum->sbuf copies split DVE/ScalarE.
  phase 2 (i-chunk outer, head inner, o-proj fused per chunk):
    per (chunk, head): score pairs kT@qT -> exp (scale fused, per-half
    offsets so only valid causal columns are computed), triangular mask on
    diagonal 128-blocks; PV matmuls interleaved one pair behind the exps;
    denominator folded on DVE (running fp16 sum of prob tiles) then one
    ones[128,128] matmul per (chunk, head) + DVE reciprocal; normalize into
    attT_c.  After 4 heads: o-projection of this chunk's 4 s-tiles
    (attT_c.T @ woT summed over heads) -> fp16 partial out rows DMA'd
    immediately, spreading the output DMA across phase 2.
"""

import math

import numpy as np

B, S, D = 2, 2048, 2048
HEADS, HEAD_DIM = 16, 128
P = 128
JC = 512          # per-core projection width (4 heads x 128)
SC = 512          # s-chunk / matmul moving width
DT = D // P       # 16 contraction tiles
NSC = S // SC     # 4 s-chunks
NST = S // P      # 16 s-tiles
HPC = 4           # heads per core
N_CORES = 8
SCALE = 1.0 / math.sqrt(HEAD_DIM)

_NC_CACHE = {}


def build_module(reps=1, phases=(1, 2, 3)):
    """Build + compile the (single-program SPMD) Bass module once.

    reps>1 repeats the whole kernel body inside one NEFF (for timing:
    differencing per-call wall times cancels the fixed dispatch overhead).
    phases: which kernel phases to include (timing experiments only);
    3 = the fused per-chunk o-projection.
    """
    phases = tuple(phases)
    key = (reps, phases)
    if key in _NC_CACHE:
        return _NC_CACHE[key]

    from contextlib import ExitStack

    import concourse.tile as tile
    from concourse import bacc
    from concourse import bass_isa
    import concourse.mybir as mybir

    f16 = mybir.dt.float16
    f32 = mybir.dt.float32
    fr = mybir.dt.float32r
    FT = mybir.ActivationFunctionType

    nc = bacc.Bacc(
        "TRN2", target_bir_lowering=False, debug=False, num_devices=N_CORES
    )

    # Host pre-swizzled partition-major layouts: one fat contiguous DRAM
    # run per SBUF partition per DMA (8-32KB descriptors instead of 2KB).
    xh = nc.dram_tensor("xh", [P, NSC, DT, SC], fr, kind="ExternalInput").ap()
    wqh = nc.dram_tensor("wqh", [P, DT, JC], fr, kind="ExternalInput").ap()
    wkh = nc.dram_tensor("wkh", [P, DT, JC], fr, kind="ExternalInput").ap()
    wvh = nc.dram_tensor("wvh", [P, DT, JC], fr, kind="ExternalInput").ap()
    woh = nc.dram_tensor("woh", [P, HPC, D], f16, kind="ExternalInput").ap()
    # mask[j, c] = 1 iff j <= c : causal triangle for a diagonal 128-block
    mask = nc.dram_tensor("mask", [P, P], f16, kind="ExternalInput").ap()
    ones = nc.dram_tensor("ones", [P, P], f16, kind="ExternalInput").ap()
    out = nc.dram_tensor("out", [S, D], f16, kind="ExternalOutput").ap()

    with tile.TileContext(nc) as tc, ExitStack() as ctx:
        consts = ctx.enter_context(tc.tile_pool(name="consts", bufs=1))
        stage = ctx.enter_context(tc.tile_pool(name="stage", bufs=2))

        mask_sb = consts.tile([P, P], f16, tag="mask", name="mask_sb")
        nc.sync.dma_start(mask_sb, mask)
        ones_sb = consts.tile([P, P], f16, tag="ones", name="ones_sb")
        nc.sync.dma_start(ones_sb, ones)

        for _rep in range(reps):
            with ExitStack() as prep:
                # resident fp16 activations: 3 x 2.1MB
                kvpool = prep.enter_context(tc.tile_pool(name="kvpool", bufs=1))
                kT_all = kvpool.tile([P, HPC, S], f16, tag="kT", name="kT_all")
                qT_all = kvpool.tile([P, HPC, S], f16, tag="qT", name="qT_all")
                v_all = kvpool.tile([P, NST, JC], f16, tag="v", name="v_all")

                # ---------- Phase 1: q/k/v projections ----------
                with ExitStack() as p1:
                    wpool = p1.enter_context(tc.tile_pool(name="wpool", bufs=1))
                    wvpool = p1.enter_context(
                        tc.tile_pool(name="wvpool", bufs=3)
                    )
                    xpool = p1.enter_context(tc.tile_pool(name="xpool", bufs=2))
                    psum1 = p1.enter_context(
                        tc.tile_pool(name="psum1", bufs=2, space="PSUM")
                    )

                    # Phase 1 stays float32r: fp16 matmuls emit a per-matmul
                    # Ldweights (unmodeled on the PE); f32r self-loads and
                    # runs the same 1 cycle/row at moving>=256.  wq/wk
                    # resident; wv streamed per dt-tile (dt-outer loop).
                    wq_sb = wpool.tile([P, DT, JC], fr, tag="wq", name="wq_sb")
                    wk_sb = wpool.tile([P, DT, JC], fr, tag="wk", name="wk_sb")

                    for sc in range(NSC):
                        xc = xpool.tile(
                            [P, DT, SC], fr, tag="x", name=f"xc_{sc}"
                        )
                        if sc == 0:
                            # finer pieces on the first chunk so the PE
                            # starts after ~1.3MB of DMA
                            for q4 in range(4):
                                dts = slice(q4 * 4, (q4 + 1) * 4)
                                nc.sync.dma_start(
                                    xc[:, dts, :], xh[:, sc, dts, :]
                                )
                        else:
                            # 2 pieces (16KB/partition runs) so one
                            # monolithic transfer never monopolizes the DMA
                            # fabric while this chunk's wv pieces are due
                            for q8 in range(2):
                                dts = slice(q8 * 8, (q8 + 1) * 8)
                                nc.sync.dma_start(
                                    xc[:, dts, :], xh[:, sc, dts, :]
                                )

                        # v projection first (dt-outer; wv streamed in 4-dt
                        # pieces every chunk)
                        ps_v = [
                            psum1.tile(
                                [P, JC], f32, tag=f"pv{t}", bufs=1, name="ps_v"
                            )
                            for t in range(4)
                        ]
                        wv_q4 = []
                        for q4 in range(4):
                            wv_p = wvpool.tile(
                                [P, 4, JC], fr, tag="wv", name="wv_p"
                            )
                            wv_q4.append(wv_p)
                            nc.scalar.dma_start(
                                wv_p, wvh[:, q4 * 4:(q4 + 1) * 4, :]
                            )
                        for dt in range(DT):
                            wv_dt = wv_q4[dt // 4][:, dt % 4, :]
                            for t in range(4):
                                # v tile [s, dv] = x_chunk.T @ wv_slice
                                nc.tensor.matmul(
                                    ps_v[t],
                                    lhsT=xc[:, dt, t * P:(t + 1) * P],
                                    rhs=wv_dt,
                                    start=(dt == 0),
                                    stop=(dt == DT - 1),
                                )
                        for t in range(4):
                            nc.vector.tensor_copy(
                                v_all[:, sc * 4 + t, :], ps_v[t]
                            )
                        if sc == 0:
                            # queue the resident k/q weights (k first: it's
                            # the next consumer); one fat DMA each
                            nc.scalar.dma_start(wk_sb, wkh)
                            nc.scalar.dma_start(wq_sb, wqh)

                        # k projection into resident kT_all (copy on ScalarE
                        # which is otherwise idle in phase 1)
                        for t in range(4):
                            ps = psum1.tile([P, SC], f32, tag="pj", name="ps_k")
                            for dt in range(DT):
                                nc.tensor.matmul(
                                    ps,
                                    lhsT=wk_sb[:, dt, t * P:(t + 1) * P],
                                    rhs=xc[:, dt, :],
                                    start=(dt == 0),
                                    stop=(dt == DT - 1),
                                )
                            nc.scalar.copy(
                                kT_all[:, t, sc * SC:(sc + 1) * SC], ps
                            )

                        # q projection into resident qT_all
                        for t in range(4):
                            ps = psum1.tile([P, SC], f32, tag="pj", name="ps_q")
                            for dt in range(DT):
                                nc.tensor.matmul(
                                    ps,
                                    lhsT=wq_sb[:, dt, t * P:(t + 1) * P],
                                    rhs=xc[:, dt, :],
                                    start=(dt == 0),
                                    stop=(dt == DT - 1),
                                )
                            nc.vector.tensor_copy(
                                qT_all[:, t, sc * SC:(sc + 1) * SC], ps
                            )

                # ---------- Phase 2: attention + fused o-proj ----------
                if 2 not in phases:
                    continue
                with ExitStack() as p2:
                    opool = p2.enter_context(tc.tile_pool(name="opool", bufs=1))
                    attp = p2.enter_context(tc.tile_pool(name="attp", bufs=2))
                    ppool = p2.enter_context(tc.tile_pool(name="ppool", bufs=5))
                    fpool = p2.enter_context(tc.tile_pool(name="fpool", bufs=2))
                    rpool = p2.enter_context(tc.tile_pool(name="rpool", bufs=2))
                    ostage = p2.enter_context(
                        tc.tile_pool(name="ostage", bufs=2)
                    )
                    psum2 = p2.enter_context(
                        tc.tile_pool(name="psum2", bufs=2, space="PSUM")
                    )

                    # o-proj weights fp16 (2.1MB), one fat DMA at phase-2
                    # entry; first needed ~4 heads later so fully hidden
                    woTs = opool.tile([P, HPC, D], f16, tag="wo", name="woTs")
                    nc.scalar.dma_start(woTs, woh)

                    for ic in range(NSC):
                        njt = 4 * ic + 4  # causal: j-tiles 0..njt-1
                        offs = [
                            max(0, (jt - 4 * ic) * P) for jt in range(njt)
                        ]
                        attT_c = attp.tile(
                            [P, HPC, SC], f16, tag="attT", name=f"attT_{ic}"
                        )
                        for h in range(HPC):
                            # scores in pairs of j-tiles -> one wide exp;
                            # PV matmuls run one pair behind the exps
                            facc = fpool.tile(
                                [P, SC], f16, tag="facc", name="facc"
                            )
                            ps_pv = psum2.tile(
                                [P, SC], f32, tag="pv", name="ps_pv"
                            )
                            pts = []  # (wide prob tile, half index u) per jt
                            for g in range(njt // 2):
                                ps_s = psum2.tile(
                                    [P, 2 * SC], f32, tag="score", bufs=2,
                                    name="ps_s",
                                )
                                ptw = ppool.tile(
                                    [P, 2 * SC], f16, tag="prob", name="pt"
                                )
                                o0, o1 = offs[2 * g], offs[2 * g + 1]
                                for u, off in ((0, o0), (1, o1)):
                                    jt = 2 * g + u
                                    pts.append((ptw, u))
                                    nc.tensor.matmul(
                                        ps_s[:, u * SC + off:(u + 1) * SC],
                                        lhsT=kT_all[
                                            :, h, jt * P:(jt + 1) * P
                                        ],
                                        rhs=qT_all[
                                            :, h,
                                            ic * SC + off:(ic + 1) * SC,
                                        ],
                                        start=True,
                                        stop=True,
                                    )
                                if o0 == o1:
                                    nc.scalar.activation(
                                        ptw[:, o0:], ps_s[:, o0:],
                                        FT.Exp, scale=SCALE,
                                    )
                                else:
                                    nc.scalar.activation(
                                        ptw[:, o0:SC], ps_s[:, o0:SC],
                                        FT.Exp, scale=SCALE,
                                    )
                                    nc.scalar.activation(
                                        ptw[:, SC + o1:],
                                        ps_s[:, SC + o1:],
                                        FT.Exp, scale=SCALE,
                                    )
                                for u, off in ((0, o0), (1, o1)):
                                    jt = 2 * g + u
                                    if jt >= 4 * ic:
                                        # triangular mask on diagonal block
                                        nc.vector.tensor_mul(
                                            out=ptw[
                                                :,
                                                u * SC + off:u * SC + off + P,
                                            ],
                                            in0=ptw[
                                                :,
                                                u * SC + off:u * SC + off + P,
                                            ],
                                            in1=mask_sb,
                                        )
                                # denominator fold on DVE (fp16, all-SBUF)
                                for u, off in ((0, o0), (1, o1)):
                                    src = ptw[:, u * SC + off:(u + 1) * SC]
                                    if g == 0 and u == 0:
                                        nc.vector.tensor_copy(facc, src)
                                    else:
                                        nc.vector.tensor_add(
                                            facc[:, off:], facc[:, off:], src
                                        )
                                # PV two pairs behind (slack for the exp
                                # latency on the ScalarE so the PE never
                                # waits on a fresh activation)
                                if g > 1:
                                    for jt in (2 * g - 4, 2 * g - 3):
                                        off = offs[jt]
                                        pw, u = pts[jt]
                                        nc.tensor.matmul(
                                            ps_pv[:, off:],
                                            lhsT=v_all[
                                                :, jt,
                                                h * HEAD_DIM:
                                                (h + 1) * HEAD_DIM,
                                            ],
                                            rhs=pw[
                                                :, u * SC + off:(u + 1) * SC
                                            ],
                                            start=(jt == 0),
                                            stop=False,
                                            skip_group_check=True,
                                        )
                            for jt in range(max(0, njt - 4), njt):
                                off = offs[jt]
                                pw, u = pts[jt]
                                nc.tensor.matmul(
                                    ps_pv[:, off:],
                                    lhsT=v_all[
                                        :, jt, h * HEAD_DIM:(h + 1) * HEAD_DIM
                                    ],
                                    rhs=pw[:, u * SC + off:(u + 1) * SC],
                                    start=(jt == 0),
                                    stop=(jt == njt - 1),
                                    skip_group_check=True,
                                )
                            # denominator: cross-partition reduce of the
                            # fold accumulator on the idle GPSIMD engine
                            # (SBUF in/out) - keeps the PE off the fold
                            # chain's critical path entirely
                            den_sb = rpool.tile(
                                [P, SC], f32, tag="den_sb", name="den_sb"
                            )
                            nc.gpsimd.partition_all_reduce(
                                den_sb, facc, P, bass_isa.ReduceOp.add
                            )
                            rec = rpool.tile(
                                [P, SC], f32, tag="rec", name="rec"
                            )
                            nc.vector.reciprocal(rec, den_sb)
                            nc.vector.tensor_mul(
                                out=attT_c[:, h, :], in0=ps_pv, in1=rec
                            )

                        # fused o-proj for this chunk's 4 s-tiles
                        if 3 not in phases:
                            continue
                        for t in range(4):
                            st = 4 * ic + t
                            og = ostage.tile([P, D], f16, tag="og", name="og")
                            for mc in range(D // SC):
                                # alternate den/po banks so each og copy
                                # hides behind the other bank's matmuls
                                ps_o = psum2.tile(
                                    [P, SC], f32,
                                    tag=("den" if mc % 2 == 0 else "po"),
                                    bufs=1, name="ps_o",
                                )
                                for hh in range(HPC):
                                    nc.tensor.matmul(
                                        ps_o,
                                        lhsT=attT_c[:, hh, t * P:(t + 1) * P],
                                        rhs=woTs[:, hh, mc * SC:(mc + 1) * SC],
                                        start=(hh == 0),
                                        stop=(hh == HPC - 1),
                                    )
                                # split og copies DVE/ScalarE: DVE is the
                                # busiest non-PE engine in phase 2
                                if mc % 2 == 0:
                                    nc.vector.tensor_copy(
                                        og[:, mc * SC:(mc + 1) * SC], ps_o
                                    )
                                else:
                                    nc.scalar.copy(
                                        og[:, mc * SC:(mc + 1) * SC], ps_o
                                    )
                            nc.sync.dma_start(
                                out[st * P:(st + 1) * P, :], og
                            )

    nc.compile()
    _NC_CACHE[key] = nc
    return nc


def make_in_maps(x, wq, wk, wv, wo):
    x = np.asarray(x, dtype=np.float32)
    wq = np.asarray(wq, dtype=np.float32)
    wk = np.asarray(wk, dtype=np.float32)
    wv = np.asarray(wv, dtype=np.float32)
    wo = np.asarray(wo, dtype=np.float32)
    # mask[j, c] = 1 iff key j visible to query c within a diagonal block
    causal = np.triu(np.ones((P, P), dtype=np.float16))
    ones = np.ones((P, P), dtype=np.float16)
    in_maps = []
    for c in range(N_CORES):
        b, g = divmod(c, HPC)
        j0 = g * JC
        # partition-major fat layouts (one contiguous DRAM run per
        # partition per DMA):
        #   xh [p, sc, dt, s] = x[b].T[(dt p), (sc s)]
        xh = np.ascontiguousarray(
            x[b].T.reshape(DT, P, NSC, SC).transpose(1, 2, 0, 3)
        )
        #   wqh [p, dt, j] = wq[j0:j0+JC].T[(dt p), j]
        wqh = np.ascontiguousarray(
            wq[j0:j0 + JC].T.reshape(DT, P, JC).transpose(1, 0, 2)
        )
        wkh = np.ascontiguousarray(
            wk[j0:j0 + JC].T.reshape(DT, P, JC).transpose(1, 0, 2)
        )
        wvh = np.ascontiguousarray(
            wv[j0:j0 + JC].T.reshape(DT, P, JC).transpose(1, 0, 2)
        )
        #   woh [p, hh, m] = wo[:, j0:j0+JC].T[(hh p), m]
        woh = np.ascontiguousarray(
            wo[:, j0:j0 + JC].T.reshape(HPC, P, D).transpose(1, 0, 2)
        ).astype(np.float16)
        in_maps.append(
            {
                "xh": xh,
                "wqh": wqh,
                "wkh": wkh,
                "wvh": wvh,
                "woh": woh,
                "mask": causal,
                "ones": ones,
            }
        )
    return in_maps


def combine_outputs(results):
    out = np.zeros((B, S, D), dtype=np.float32)
    for c in range(N_CORES):
        out[c // HPC] += np.asarray(results[c]["out"], dtype=np.float32)
    return out


def kernel(x, wq, wk, wv, wo):
    from concourse.bass_utils import run_bass_kernel_spmd

    nc = build_module()
    in_maps = make_in_maps(x, wq, wk, wv, wo)
    res = run_bass_kernel_spmd(nc, in_maps, list(range(N_CORES)))
    return combine_outputs(res.results)


# revision 31
# speedup vs baseline: 1.0991x; 1.0991x over previous
"""Multi-head causal attention (B=2, S=2048, D=2048, 16 heads) on 8 TRN2 cores.

Sharding: 2-way batch parallel x 4-way head tensor-parallel (4 heads/core).
Each core computes q/k/v projections for its 4 heads, causal softmax
attention, and a partial o-projection; the host sums the 4 partials per batch.

v4: float32r matmuls in phase 1 and the o-projection (f32r self-loads its
stationary; 2-byte dtypes emit a per-matmul Ldweights the PE pays serially
on HW), fp16 for the attention phase + all resident activations (q/k/v stay
fully resident - no DRAM spill - and the out DMA is halved).  Host
pre-swizzles every input into a partition-major layout so each DMA is one
fat (8-32KB) contiguous DRAM run per SBUF partition:
  xh  [p, sc, dt, s] = x[b].T[(dt p), (sc s)]
  wqh [p, dt, j]     = wq[j0:j0+512].T[(dt p), j]   (same wkh, wvh)
  woh [p, hh, m]     = wo[:, j0:j0+512].T[(hh p), m]

On-chip dataflow (per core):
  phase 1: kT_all[dk,h,s], qT_all[dk,h,s], v_all[s,st,dv] all resident fp16.
           wv streamed per chunk in 4-dt pieces (dt-outer loop, 4 parallel
           PSUM banks) so the PE starts after ~1.3MB of DMA; wq/wk resident,
           k/q projections s-outer, psum->sbuf copies split DVE/ScalarE.
  phase 2 (i-chunk outer, head inner, o-proj fused per chunk):
    per (chunk, head): score pairs kT@qT -> exp (scale fused, per-half
    offsets so only valid causal columns are computed), triangular mask on
    diagonal 128-blocks; PV matmuls interleaved one pair behind the exps;
    denominator folded on DVE (running fp16 sum of prob tiles) then one
    ones[128,128] matmul per (chunk, head) + DVE reciprocal; normalize into
    attT_c.  After 4 heads: o-projection of this chunk's 4 s-tiles
    (attT_c.T @ woT summed over heads) -> fp16 partial out rows DMA'd
    immediately, spreading the output DMA across phase 2.
"""

import math

import numpy as np

B, S, D = 2, 2048, 2048
HEADS, HEAD_DIM = 16, 128
P = 128
JC = 512          # per-core projection width (4 heads x 128)
SC = 512          # s-chunk / matmul moving width
DT = D // P       # 16 contraction tiles
NSC = S // SC     # 4 s-chunks
NST = S // P      # 16 s-tiles
HPC = 4           # heads per core
N_CORES = 8
SCALE = 1.0 / math.sqrt(HEAD_DIM)

_NC_CACHE = {}


def build_module(reps=1, phases=(1, 2, 3)):
    """Build + compile the (single-program SPMD) Bass module once.

    reps>1 repeats the whole kernel body inside one NEFF (for timing:
    differencing per-call wall times cancels the fixed dispatch overhead).
    phases: which kernel phases to include (timing experiments only);
    3 = the fused per-chunk o-projection.
    """
    phases = tuple(phases)
    key = (reps, phases)
    if key in _NC_CACHE:
        return _NC_CACHE[key]

    from contextlib import ExitStack

    import concourse.tile as tile
    from concourse import bacc
    import concourse.mybir as mybir

    f16 = mybir.dt.float16
    f32 = mybir.dt.float32
    fr = mybir.dt.float32r
    FT = mybir.ActivationFunctionType

    nc = bacc.Bacc(
        "TRN2", target_bir_lowering=False, debug=False, num_devices=N_CORES
    )

    # Host pre-swizzled partition-major layouts: one fat contiguous DRAM
    # run per SBUF partition per DMA (8-32KB descriptors instead of 2KB).
    xh = nc.dram_tensor("xh", [P, NSC, DT, SC], fr, kind="ExternalInput").ap()
    wqh = nc.dram_tensor("wqh", [P, DT, JC], fr, kind="ExternalInput").ap()
    wkh = nc.dram_tensor("wkh", [P, DT, JC], fr, kind="ExternalInput").ap()
    wvh = nc.dram_tensor("wvh", [P, DT, JC], fr, kind="ExternalInput").ap()
    woh = nc.dram_tensor("woh", [P, HPC, D], f16, kind="ExternalInput").ap()
    # mask[j, c] = 1 iff j <= c : causal triangle for a diagonal 128-block
    mask = nc.dram_tensor("mask", [P, P], f16, kind="ExternalInput").ap()
    ones = nc.dram_tensor("ones", [P, P], f16, kind="ExternalInput").ap()
    out = nc.dram_tensor("out", [S, D], f16, kind="ExternalOutput").ap()

    with tile.TileContext(nc) as tc, ExitStack() as ctx:
        consts = ctx.enter_context(tc.tile_pool(name="consts", bufs=1))
        stage = ctx.enter_context(tc.tile_pool(name="stage", bufs=2))

        mask_sb = consts.tile([P, P], f16, tag="mask", name="mask_sb")
        nc.sync.dma_start(mask_sb, mask)
        ones_sb = consts.tile([P, P], f16, tag="ones", name="ones_sb")
        nc.sync.dma_start(ones_sb, ones)

        for _rep in range(reps):
            with ExitStack() as prep:
                # resident fp16 activations: 3 x 2.1MB
                kvpool = prep.enter_context(tc.tile_pool(name="kvpool", bufs=1))
                kT_all = kvpool.tile([P, HPC, S], f16, tag="kT", name="kT_all")
                qT_all = kvpool.tile([P, HPC, S], f16, tag="qT", name="qT_all")
                v_all = kvpool.tile([P, NST, JC], f16, tag="v", name="v_all")

                # ---------- Phase 1: q/k/v projections ----------
                with ExitStack() as p1:
                    wpool = p1.enter_context(tc.tile_pool(name="wpool", bufs=1))
                    wvpool = p1.enter_context(
                        tc.tile_pool(name="wvpool", bufs=3)
                    )
                    xpool = p1.enter_context(tc.tile_pool(name="xpool", bufs=2))
                    psum1 = p1.enter_context(
                        tc.tile_pool(name="psum1", bufs=2, space="PSUM")
                    )

                    # Phase 1 stays float32r: fp16 matmuls emit a per-matmul
                    # Ldweights (unmodeled on the PE); f32r self-loads and
                    # runs the same 1 cycle/row at moving>=256.  wq/wk
                    # resident; wv streamed per dt-tile (dt-outer loop).
                    wq_sb = wpool.tile([P, DT, JC], fr, tag="wq", name="wq_sb")
                    wk_sb = wpool.tile([P, DT, JC], fr, tag="wk", name="wk_sb")

                    for sc in range(NSC):
                        xc = xpool.tile(
                            [P, DT, SC], fr, tag="x", name=f"xc_{sc}"
                        )
                        if sc == 0:
                            # finer pieces on the first chunk so the PE
                            # starts after ~1.3MB of DMA
                            for q4 in range(4):
                                dts = slice(q4 * 4, (q4 + 1) * 4)
                                nc.sync.dma_start(
                                    xc[:, dts, :], xh[:, sc, dts, :]
                                )
                        else:
                            # 2 pieces (16KB/partition runs) so one
                            # monolithic transfer never monopolizes the DMA
                            # fabric while this chunk's wv pieces are due
                            for q8 in range(2):
                                dts = slice(q8 * 8, (q8 + 1) * 8)
                                nc.sync.dma_start(
                                    xc[:, dts, :], xh[:, sc, dts, :]
                                )

                        # v projection first (dt-outer; wv streamed in 4-dt
                        # pieces every chunk)
                        ps_v = [
                            psum1.tile(
                                [P, JC], f32, tag=f"pv{t}", bufs=1, name="ps_v"
                            )
                            for t in range(4)
                        ]
                        wv_q4 = []
                        for q4 in range(4):
                            wv_p = wvpool.tile(
                                [P, 4, JC], fr, tag="wv", name="wv_p"
                            )
                            wv_q4.append(wv_p)
                            nc.scalar.dma_start(
                                wv_p, wvh[:, q4 * 4:(q4 + 1) * 4, :]
                            )
                        for dt in range(DT):
                            wv_dt = wv_q4[dt // 4][:, dt % 4, :]
                            for t in range(4):
                                # v tile [s, dv] = x_chunk.T @ wv_slice
                                nc.tensor.matmul(
                                    ps_v[t],
                                    lhsT=xc[:, dt, t * P:(t + 1) * P],
                                    rhs=wv_dt,
                                    start=(dt == 0),
                                    stop=(dt == DT - 1),
                                )
                        for t in range(4):
                            nc.vector.tensor_copy(
                                v_all[:, sc * 4 + t, :], ps_v[t]
                            )
                        if sc == 0:
                            # resident k/q weights on the otherwise-idle
                            # gpsimd queue (k first: it's the next consumer)
                            # so the scalar queue stays a pure wv stream and
                            # chunk 1's wv pieces are never stuck behind them
                            nc.gpsimd.dma_start(wk_sb, wkh)
                            nc.gpsimd.dma_start(wq_sb, wqh)

                        # k projection into resident kT_all (copy on ScalarE
                        # which is otherwise idle in phase 1)
                        for t in range(4):
                            ps = psum1.tile([P, SC], f32, tag="pj", name="ps_k")
                            for dt in range(DT):
                                nc.tensor.matmul(
                                    ps,
                                    lhsT=wk_sb[:, dt, t * P:(t + 1) * P],
                                    rhs=xc[:, dt, :],
                                    start=(dt == 0),
                                    stop=(dt == DT - 1),
                                )
                            nc.scalar.copy(
                                kT_all[:, t, sc * SC:(sc + 1) * SC], ps
                            )

                        # q projection into resident qT_all
                        for t in range(4):
                            ps = psum1.tile([P, SC], f32, tag="pj", name="ps_q")
                            for dt in range(DT):
                                nc.tensor.matmul(
                                    ps,
                                    lhsT=wq_sb[:, dt, t * P:(t + 1) * P],
                                    rhs=xc[:, dt, :],
                                    start=(dt == 0),
                                    stop=(dt == DT - 1),
                                )
                            nc.vector.tensor_copy(
                                qT_all[:, t, sc * SC:(sc + 1) * SC], ps
                            )

                # ---------- Phase 2: attention + fused o-proj ----------
                if 2 not in phases:
                    continue
                with ExitStack() as p2:
                    opool = p2.enter_context(tc.tile_pool(name="opool", bufs=1))
                    attp = p2.enter_context(tc.tile_pool(name="attp", bufs=2))
                    ppool = p2.enter_context(tc.tile_pool(name="ppool", bufs=6))
                    fpool = p2.enter_context(tc.tile_pool(name="fpool", bufs=2))
                    rpool = p2.enter_context(tc.tile_pool(name="rpool", bufs=2))
                    ostage = p2.enter_context(
                        tc.tile_pool(name="ostage", bufs=2)
                    )
                    psum2 = p2.enter_context(
                        tc.tile_pool(name="psum2", bufs=2, space="PSUM")
                    )

                    # o-proj weights fp16 (2.1MB), one fat DMA at phase-2
                    # entry; first needed ~4 heads later so fully hidden
                    woTs = opool.tile([P, HPC, D], f16, tag="wo", name="woTs")
                    nc.scalar.dma_start(woTs, woh)

                    # biggest chunk first: its deep score/pv pipeline warms
                    # phase 2 and hides the woTs load (chunks independent)
                    for ic in range(NSC - 1, -1, -1):
                        njt = 4 * ic + 4  # causal: j-tiles 0..njt-1
                        offs = [
                            max(0, (jt - 4 * ic) * P) for jt in range(njt)
                        ]
                        attT_c = attp.tile(
                            [P, HPC, SC], f16, tag="attT", name=f"attT_{ic}"
                        )
                        for h in range(HPC):
                            # scores in pairs of j-tiles -> one wide exp;
                            # PV matmuls run one pair behind the exps
                            facc = fpool.tile(
                                [P, SC], f16, tag="facc", name="facc"
                            )
                            ps_pv = psum2.tile(
                                [P, SC], f32, tag="pv", name="ps_pv"
                            )
                            pts = []  # (wide prob tile, half index u) per jt
                            for g in range(njt // 2):
                                ps_s = psum2.tile(
                                    [P, 2 * SC], f32, tag="score", bufs=2,
                                    name="ps_s",
                                )
                                ptw = ppool.tile(
                                    [P, 2 * SC], f16, tag="prob", name="pt"
                                )
                                o0, o1 = offs[2 * g], offs[2 * g + 1]
                                for u, off in ((0, o0), (1, o1)):
                                    jt = 2 * g + u
                                    pts.append((ptw, u))
                                    nc.tensor.matmul(
                                        ps_s[:, u * SC + off:(u + 1) * SC],
                                        lhsT=kT_all[
                                            :, h, jt * P:(jt + 1) * P
                                        ],
                                        rhs=qT_all[
                                            :, h,
                                            ic * SC + off:(ic + 1) * SC,
                                        ],
                                        start=True,
                                        stop=True,
                                    )
                                if o0 == o1:
                                    nc.scalar.activation(
                                        ptw[:, o0:], ps_s[:, o0:],
                                        FT.Exp, scale=SCALE,
                                    )
                                else:
                                    nc.scalar.activation(
                                        ptw[:, o0:SC], ps_s[:, o0:SC],
                                        FT.Exp, scale=SCALE,
                                    )
                                    nc.scalar.activation(
                                        ptw[:, SC + o1:],
                                        ps_s[:, SC + o1:],
                                        FT.Exp, scale=SCALE,
                                    )
                                for u, off in ((0, o0), (1, o1)):
                                    jt = 2 * g + u
                                    if jt >= 4 * ic:
                                        # triangular mask on diagonal block
                                        nc.vector.tensor_mul(
                                            out=ptw[
                                                :,
                                                u * SC + off:u * SC + off + P,
                                            ],
                                            in0=ptw[
                                                :,
                                                u * SC + off:u * SC + off + P,
                                            ],
                                            in1=mask_sb,
                                        )
                                # denominator fold on DVE (fp16, all-SBUF)
                                for u, off in ((0, o0), (1, o1)):
                                    src = ptw[:, u * SC + off:(u + 1) * SC]
                                    if g == 0 and u == 0:
                                        nc.vector.tensor_copy(facc, src)
                                    else:
                                        nc.vector.tensor_add(
                                            facc[:, off:], facc[:, off:], src
                                        )
                                # PV two pairs behind (slack for the exp
                                # latency on the ScalarE so the PE never
                                # waits on a fresh activation)
                                if g > 1:
                                    for jt in (2 * g - 4, 2 * g - 3):
                                        off = offs[jt]
                                        pw, u = pts[jt]
                                        nc.tensor.matmul(
                                            ps_pv[:, off:],
                                            lhsT=v_all[
                                                :, jt,
                                                h * HEAD_DIM:
                                                (h + 1) * HEAD_DIM,
                                            ],
                                            rhs=pw[
                                                :, u * SC + off:(u + 1) * SC
                                            ],
                                            start=(jt == 0),
                                            stop=False,
                                            skip_group_check=True,
                                        )
                            for jt in range(max(0, njt - 4), njt):
                                off = offs[jt]
                                pw, u = pts[jt]
                                nc.tensor.matmul(
                                    ps_pv[:, off:],
                                    lhsT=v_all[
                                        :, jt, h * HEAD_DIM:(h + 1) * HEAD_DIM
                                    ],
                                    rhs=pw[:, u * SC + off:(u + 1) * SC],
                                    start=(jt == 0),
                                    stop=(jt == njt - 1),
                                    skip_group_check=True,
                                )
                            # denominator: one matmul per (chunk, head)
                            ps_den = psum2.tile(
                                [P, SC], f32, tag="den", bufs=1, name="ps_den"
                            )
                            nc.tensor.matmul(
                                ps_den, lhsT=ones_sb, rhs=facc,
                                start=True, stop=True,
                            )
                            rec = rpool.tile(
                                [P, SC], f32, tag="rec", name="rec"
                            )
                            nc.vector.reciprocal(rec, ps_den)
                            nc.vector.tensor_mul(
                                out=attT_c[:, h, :], in0=ps_pv, in1=rec
                            )

                        # fused o-proj for this chunk's 4 s-tiles
                        if 3 not in phases:
                            continue
                        for t in range(4):
                            st = 4 * ic + t
                            og = ostage.tile([P, D], f16, tag="og", name="og")
                            for mc in range(D // SC):
                                # alternate den/po banks so each og copy
                                # hides behind the other bank's matmuls
                                ps_o = psum2.tile(
                                    [P, SC], f32,
                                    tag=("den" if mc % 2 == 0 else "po"),
                                    bufs=1, name="ps_o",
                                )
                                for hh in range(HPC):
                                    nc.tensor.matmul(
                                        ps_o,
                                        lhsT=attT_c[:, hh, t * P:(t + 1) * P],
                                        rhs=woTs[:, hh, mc * SC:(mc + 1) * SC],
                                        start=(hh == 0),
                                        stop=(hh == HPC - 1),
                                    )
                                # split og copies DVE/ScalarE: DVE is the
                                # busiest non-PE engine in phase 2
                                if mc % 2 == 0:
                                    nc.vector.tensor_copy(
                                        og[:, mc * SC:(mc + 1) * SC], ps_o
                                    )
                                else:
                                    nc.scalar.copy(
                                        og[:, mc * SC:(mc + 1) * SC], ps_o
                                    )
                            nc.sync.dma_start(
                                out[st * P:(st + 1) * P, :], og
                            )

    nc.compile()
    _NC_CACHE[key] = nc
    return nc


def make_in_maps(x, wq, wk, wv, wo):
    x = np.asarray(x, dtype=np.float32)
    wq = np.asarray(wq, dtype=np.float32)
    wk = np.asarray(wk, dtype=np.float32)
    wv = np.asarray(wv, dtype=np.float32)
    wo = np.asarray(wo, dtype=np.float32)
    # mask[j, c] = 1 iff key j visible to query c within a diagonal block
    causal = np.triu(np.ones((P, P), dtype=np.float16))
    ones = np.ones((P, P), dtype=np.float16)
    in_maps = []
    for c in range(N_CORES):
        b, g = divmod(c, HPC)
        j0 = g * JC
        # partition-major fat layouts (one contiguous DRAM run per
        # partition per DMA):
        #   xh [p, sc, dt, s] = x[b].T[(dt p), (sc s)]
        xh = np.ascontiguousarray(
            x[b].T.reshape(DT, P, NSC, SC).transpose(1, 2, 0, 3)
        )
        #   wqh [p, dt, j] = wq[j0:j0+JC].T[(dt p), j]
        wqh = np.ascontiguousarray(
            wq[j0:j0 + JC].T.reshape(DT, P, JC).transpose(1, 0, 2)
        )
        wkh = np.ascontiguousarray(
            wk[j0:j0 + JC].T.reshape(DT, P, JC).transpose(1, 0, 2)
        )
        wvh = np.ascontiguousarray(
            wv[j0:j0 + JC].T.reshape(DT, P, JC).transpose(1, 0, 2)
        )
        #   woh [p, hh, m] = wo[:, j0:j0+JC].T[(hh p), m]
        woh = np.ascontiguousarray(
            wo[:, j0:j0 + JC].T.reshape(HPC, P, D).transpose(1, 0, 2)
        ).astype(np.float16)
        in_maps.append(
            {
                "xh": xh,
                "wqh": wqh,
                "wkh": wkh,
                "wvh": wvh,
                "woh": woh,
                "mask": causal,
                "ones": ones,
            }
        )
    return in_maps


def combine_outputs(results):
    out = np.zeros((B, S, D), dtype=np.float32)
    for c in range(N_CORES):
        out[c // HPC] += np.asarray(results[c]["out"], dtype=np.float32)
    return out


def kernel(x, wq, wk, wv, wo):
    from concourse.bass_utils import run_bass_kernel_spmd

    nc = build_module()
    in_maps = make_in_maps(x, wq, wk, wv, wo)
    res = run_bass_kernel_spmd(nc, in_maps, list(range(N_CORES)))
    return combine_outputs(res.results)
